# revision 19
# baseline (speedup 1.0000x reference)
"""Correlation kernel for Trainium2 (Bass/Tile), 8 NeuronCores.

Problem: inputs (B=4, N=2, C=128, H=128, W=128) fp32.
  src = inputs[:, 0], target = inputs[:, 1]
  out[b, k, y, x] = (1/C) * sum_c src[b,c,y,x] * target[b,c,y+dy,x+dx]
  for k = (dy+10)*21 + (dx+10), dy,dx in [-10,10], zero-padded target.
  Output (4, 441, 128, 128) fp32.

Mapping (v2, 2D-patch matmuls + quad-compacted output):
  - Shard over 8 cores: (b in 0..3) x (H half in 0..1); 64 rows/core.
  - Per core, pixels are tiled into 64 patches of 16(y) x 8(x) = 128
    pixels. One patch = one stationary lhsT (C=128 x 128 pixels, full
    PE array; src is host-pre-tiled so the 128 pixels are a contiguous
    1D free dim). The moving rhs is the target window for the whole
    patch: 36 rows (16+2*10) x 28 cols (8+2*10) = 1008 columns, split
    into two N=504 matmuls (one PSUM bank each). Pixel m's matmul row
    holds its full 36x28 window of correlations.
  - Output compaction: pixel (py,px) only needs window rows
    t in [py, py+21). Partitions are py-major (m = py*8+px), so a
    py-quad (4 py values = 32 partitions) shares t in [4q, 4q+24) - a
    legal partition-block-uniform slice. Per band (16 patches) we DMA
    out 4 quads x 2 bx-halves: 24x28=672 of the 1008 values per pixel
    (11.0 MB/core instead of 16.5). The host extracts the final 21x21
    per pixel while unsharding.
  - Everything fp16: inputs host-pre-scaled by 2^-4/2^-3 (exact; folds
    the 1/C=2^-7 mean), PE accumulates fp32, PSUM->SBUF evacuation
    downcasts to fp16, rotating over DVE / ACT / GPSIMD so no single
    engine is critical. Total DMA ~16.2 MB/core vs 46.6 MB for the
    strip-mined baseline; PE streams 1008 cols/patch (4.3x fewer).
"""

import numpy as np

import concourse.bacc as bacc
import concourse.bass as bass
import concourse.mybir as mybir
import concourse.tile as tile
from concourse.bass_utils import run_bass_kernel_spmd

B = 4
C = 128
H = 128
W = 128
KS = 21          # kernel size (per axis)
P = KS // 2      # pad / max displacement = 10
HY = H // 2      # rows per core = 64
PY = 16          # patch rows
PX = 8           # patch cols (PY*PX = 128 = M)
TH = PY + 2 * P  # 36: target row window per patch
XW = PX + 2 * P  # 28: target col window per patch
NBY = HY // PY   # 4 bands
NBX = W // PX    # 16 x-chunks
NPATCH = NBY * NBX   # 64 patches per core
WINF = TH * XW       # 1008 window values per pixel
NSPL = 2             # matmul N-split (504 <= 512 psum bank)
TSPL = TH // NSPL    # 18 t-rows per matmul
TGT_H = HY + 2 * P   # 84 target rows per core
TGT_W = W + 2 * P    # 148 padded target width
NQ = 4               # py-quads per band
QPY = PY // NQ       # 4 py rows per quad (32 partitions)
TQ = KS + QPY - 1    # 24: t-rows shipped per quad

_CACHE = {}


def _build_module(mode: str):
    """Build the SPMD Bass module (same program on all 8 cores)."""
    f32 = mybir.dt.float32
    f16 = mybir.dt.float16
    nc = bacc.Bacc("TRN2", target_bir_lowering=False, debug=False)

    # src is pre-tiled on the host to [C, patch, pixel] so each patch's
    # 128 pixels are one contiguous free dim (stationary APs must be 1D)
    src_d = nc.declare_dram_parameter("src", [C, NPATCH, PY * PX], f16,
                                      isOutput=False)
    tgt_d = nc.declare_dram_parameter("tgt", [C, TGT_H, TGT_W], f16,
                                      isOutput=False)
    # t-major, bx-halved shipped layout: a quad's slice [32 part, half h,
    # t 4q:4q+24, 8 bx, 28 x'] is contiguous per partition; descriptors
    # split to 5376B (the ~26 B/ns DMA sweet spot measured on HW)
    out_d = nc.declare_dram_parameter("out_win", [NBY, NQ, 2, 32, TQ, 8, XW],
                                      f16, isOutput=True)

    with tile.TileContext(nc) as tc:
        with (
            tc.tile_pool(name="inp", bufs=1) as inp,
            tc.tile_pool(name="psum", bufs=4, space=bass.MemorySpace.PSUM) as psum,
            tc.tile_pool(name="win", bufs=2) as winp,
        ):
            src_sb = inp.tile([C, NPATCH, PY * PX], f16, name="sb_src")
            tgt_sb = inp.tile([C, TGT_H, TGT_W], f16, name="sb_tgt")
            # Chunked loads, smallest-deps-first so band 0 starts early.
            tgt_rows = [(0, 12), (12, 24), (24, 36), (36, 48), (48, 60),
                        (60, 72), (72, 84)]
            src_chunks = [(0, 8), (8, 16), (16, 32), (32, 48), (48, 64)]
            order = [("t", 0), ("t", 1), ("s", 0), ("t", 2), ("s", 1),
                     ("t", 3), ("s", 2), ("t", 4), ("s", 3), ("t", 5),
                     ("s", 4), ("t", 6)]
            for kind, i in order:
                if kind == "t":
                    lo, hi = tgt_rows[i]
                    nc.sync.dma_start(tgt_sb[:, lo:hi, :], tgt_d[:, lo:hi, :])
                else:
                    lo, hi = src_chunks[i]
                    nc.sync.dma_start(src_sb[:, lo:hi, :], src_d[:, lo:hi, :])

            # evac engine rotation (GPSIMD cannot access PSUM)
            def evac(i, dst, src):
                if i % 2 == 0:
                    nc.scalar.copy(dst, src)
                else:
                    nc.vector.tensor_copy(dst, src)

            for by in range(NBY):
                win = winp.tile([128, 2, TH, 8, XW], f16)
                for bx in range(NBX):
                    p = by * NBX + bx
                    ps = psum.tile([128, NSPL, 512], f32)
                    lhsT = src_sb[:, p, :]
                    for k in range(NSPL):
                        rhs = tgt_sb[:, by * PY + k * TSPL:
                                     by * PY + (k + 1) * TSPL,
                                     bx * PX: bx * PX + XW]
                        nc.tensor.matmul(
                            ps[:, k, 0:TSPL * XW],
                            lhsT, rhs, start=True, stop=True,
                        )
                    evac(p, win[:, bx // 8, :, bx % 8, :],
                         ps[:, :, 0:TSPL * XW])
                    if bx % 8 == 7:
                        h = bx // 8
                        for q in range(NQ):
                            # pre-split the 10752B per-partition run into two
                            # 5376B descriptors (the HW DMA sweet spot)
                            sb = win[32 * q:32 * q + 32, h,
                                     4 * q:4 * q + TQ, :, :]
                            nc.sync.dma_start(
                                out_d[by, q, h].rearrange(
                                    "m (s t) b x -> m s (t b x)", s=2),
                                sb.rearrange("p (s t) b x -> p s (t b x)",
                                             s=2),
                            )

    nc.compile()
    return nc


def _get_module(mode: str):
    if mode not in _CACHE:
        _CACHE[mode] = _build_module(mode)
    return _CACHE[mode]


def _shard_inputs(inputs: np.ndarray, mode: str):
    # fold the 1/C = 2^-7 mean into the inputs as 2^-3 * 2^-4 (exact,
    # and keeps both operands well inside fp16 normal range)
    src = (inputs[:, 0] * np.float32(0.125)).astype(np.float16)
    tgt = (inputs[:, 1] * np.float32(0.0625)).astype(np.float16)
    tgt_pad = np.pad(tgt, ((0, 0), (0, 0), (P, P), (P, P)))
    in_maps = []
    for core in range(8):
        b, h = divmod(core, 2)
        s = src[b, :, h * HY:(h + 1) * HY, :]
        # pre-tile to [C, patch=(by,bx), pixel=(py,px)]
        s = (s.reshape(C, NBY, PY, NBX, PX).transpose(0, 1, 3, 2, 4)
             .reshape(C, NPATCH, PY * PX))
        s = np.ascontiguousarray(s)
        t = np.ascontiguousarray(tgt_pad[b, :, h * HY: h * HY + TGT_H, :])
        in_maps.append({"src": s, "tgt": t})
    return in_maps


# flat shipped-window index for in-quad pixel m=(py_r,px), k=(dy,dx):
# t_in = py_r + (dy+10), x' = px + (dx+10); the arange(KS) axes below
# ARE (dy+10) and (dx+10)  -> shape (32, 441)
_pyr = np.arange(QPY)
_pxv = np.arange(PX)
_dv = np.arange(KS)
_FLAT = ((_pyr[:, None, None, None] + _dv[None, None, :, None]) * XW
         + _pxv[None, :, None, None] + _dv[None, None, None, :]
         ).reshape(32, KS * KS)


def _extract(win: np.ndarray) -> np.ndarray:
    """(NBY, NQ, 2, 32, TQ, 8, XW) shipped windows -> (441, HY, W) block."""
    w = np.ascontiguousarray(win.transpose(0, 1, 3, 2, 5, 4, 6))
    w = w.reshape(NBY, NQ, 32, NBX, TQ * XW)
    g = np.take_along_axis(w, _FLAT[None, None, :, None, :], axis=4)
    arr = g.reshape(NBY, NQ, QPY, PX, NBX, KS * KS)
    return (arr.transpose(5, 0, 1, 2, 4, 3)
            .reshape(KS * KS, HY, W).astype(np.float32))


def run(inputs: np.ndarray, trace: bool = False, mode: str | None = None):
    mode = "v2"
    nc = _get_module(mode)
    in_maps = _shard_inputs(inputs, mode)
    res = run_bass_kernel_spmd(
        nc, in_maps, core_ids=list(range(8)), trace=trace,
    )
    out = np.empty((B, KS * KS, H, W), dtype=np.float32)
    for core in range(8):
        b, h = divmod(core, 2)
        out[b, :, h * HY:(h + 1) * HY, :] = _extract(res.results[core]["out_win"])
    return out, res.exec_time_ns


def kernel(inputs: np.ndarray) -> np.ndarray:
    out, _ = run(np.asarray(inputs))
    return out
